# revision 8
# baseline (speedup 1.0000x reference)
"""Multi-head attention (16 heads, B=4, S=2048, E=1024) on 8 Trainium2 cores.

Sharding: core c handles batch b = c//2 and the s-half (c%2) of the query
rows — 1024 query rows per core, all 16 heads.  K/V are computed for the
full sequence of the core's batch on-chip (small recompute, no collectives).

Orientation: everything is kept transposed ([t, s] / [d, s] / [hd, s]) so
that every matmul contraction lands on the partition axis:
  K^T[d,t], Q^T[d,s]  -> scores^T[t,s] (PE, 2 heads row-tiled)
  exp on ACT (scale=1/sqrt(D) folded in) -> P^T bf16
  AV: lhsT = V|ones [t, 65] -> psum[0:64]=head_out^T raw, psum[64]=denom
  denom -> reciprocal (DVE) -> broadcast over partitions via ones-matmul (PE)
  wei^T = P^T * recip  (DVE, per-tile)  -> DMA out (bf16, host transposes)
  out   = cat^T.T @ Wo + ones.T @ bo    (PE accumulation, bias folded in)

The kernel returns (out, wei) matching reference.py; wei is written to HBM
transposed ([h, t, s_local] bf16) and is transposed/cast on the host during
unsharding.
"""

import numpy as np
import ml_dtypes

import concourse.bass as bass
import concourse.mybir as mybir
from concourse.tile import TileContext
from concourse import bass_utils

BF16 = mybir.dt.bfloat16
F32 = mybir.dt.float32

# ----------------------------------------------------------------------------
# Workarounds for this container's pinned toolchain.
# ----------------------------------------------------------------------------


def _patch_tile_drain():
    """walrus here rejects >1 sync-wait on the Tile tail drain (CTRL encoding
    limit).  Split the waits across several drain instructions on the sync
    engine — they serialize, so semantics are unchanged."""
    import concourse.tile as ctile
    from concourse.vector_clock import ScopedClock

    if getattr(ctile.TileContext, "_drain_split_patched", False):
        return

    def _drain_and_barrier_split(self, tick_clock, wait_clock):
        drain_inst = self.nc.sync.drain()
        wait_clock.add_sem_waits(
            drain_inst.ins, ScopedClock({None: tick_clock.global_clock})
        )
        waits = drain_inst.ins.sync_info.on_wait
        if len(waits) > 1:
            extra = list(waits[1:])
            del waits[1:]
            for sw in extra:
                nxt = self.nc.sync.drain()
                nxt.ins.sync_info = mybir.SyncInfo(on_wait=[sw], on_update=[])

        self.nc.all_engine_barrier()
        assert self.sems is not None
        popped = self.nc._tile_sem_poison_stack.pop()
        assert popped is self._sem_poison
        self.nc.clear_and_free_semaphores(list(self.sems.allocated().values()))
        self.nc.all_engine_barrier()

    ctile.TileContext._drain_and_barrier = _drain_and_barrier_split
    ctile.TileContext._drain_split_patched = True


_patch_tile_drain()


def _split_multi_waits(nc):
    """This walrus build accepts at most one sync-wait per instruction.
    Hoist extra waits onto single-wait drain carriers inserted immediately
    before the instruction on the same engine — identical blocking
    semantics, engine order preserved."""
    nop_instr = [159, 16] + [0] * 62  # ENGINE_NOP 64B encoding
    nop_dict = {"header": {"opcode": 159, "inst_word_len": 16}}
    k = 0
    for f in nc.m.functions:
        for bb in f.blocks:
            new_list = []
            for inst in bb.instructions:
                si = getattr(inst, "sync_info", None)
                if si is not None and si.on_wait and len(si.on_wait) > 1:
                    use_nop = inst.engine in (
                        mybir.EngineType.DVE,
                        mybir.EngineType.Pool,
                    )
                    use_nop164 = inst.engine == mybir.EngineType.Activation
                    waits = list(si.on_wait)
                    for sw in waits[:-1]:
                        k += 1
                        if use_nop or use_nop164:
                            opv = 159 if use_nop else 164
                            carrier = mybir.InstISA(
                                name=f"wsplit-{k}",
                                isa_opcode=opv,
                                op_name="ENGINE_NOP" if opv == 159 else "NOP",
                                instr=[opv, 16] + [0] * 62,
                                ant_dict={"header": {"opcode": opv, "inst_word_len": 16}},
                                ant_isa_is_sequencer_only=False,
                                engine=inst.engine,
                                ins=[],
                                outs=[],
                                sync_info=mybir.SyncInfo(
                                    on_wait=[sw], on_update=[]
                                ),
                            )
                        else:
                            carrier = mybir.InstDrain(
                                name=f"wsplit-{k}",
                                engine=inst.engine,
                                ins=[],
                                outs=[],
                                sync_info=mybir.SyncInfo(
                                    on_wait=[sw], on_update=[]
                                ),
                            )
                        new_list.append(carrier)
                    inst.sync_info = mybir.SyncInfo(
                        on_wait=[waits[-1]],
                        on_update=list(si.on_update or []),
                    )
                new_list.append(inst)
            bb.instructions = new_list
    return k


# ----------------------------------------------------------------------------
# Per-core Bass program
# ----------------------------------------------------------------------------


def build_core_program(E=1024, H=16, S=2048, SL=1024, split_waits=True):
    """Emit the per-core MHA program.  All cores run this same program on
    different input data (SPMD).  D is fixed at 64.

    Per-core inputs (bf16):
      xT   [E, S]   - x[b].T, full sequence of this core's batch
      xTq  [E, SL]  - local query columns of xT
      wq/wk/wv [E, H*64] - per-head projection weights, hd = h*64+d
      wo   [H*64, E]
      bo   [1, E]
    Outputs:
      weiT [H, S, SL] bf16 - attention map, transposed in last two dims
      out  [SL, E] f32
    """
    D = 64
    HD = H * D
    EC = E // 128          # e-chunks (contraction blocks for projections)
    NP = H // 2            # head pairs
    T128 = S // 128        # key/t blocks of 128
    TW = min(512, S)       # t width for K^T production
    SW = min(512, SL)      # s width for scores / attention blocks
    NB = max(1, HD // 512) # 512-wide chunks of HD for the V projection
    VW = min(512, HD)

    nc = bass.Bass(trn_type="TRN2")

    xT = nc.dram_tensor("xT", [E, S], BF16, kind="ExternalInput")
    xTq = nc.dram_tensor("xTq", [E, SL], BF16, kind="ExternalInput")
    wq = nc.dram_tensor("wq", [E, HD], BF16, kind="ExternalInput")
    wk = nc.dram_tensor("wk", [E, HD], BF16, kind="ExternalInput")
    wv = nc.dram_tensor("wv", [E, HD], BF16, kind="ExternalInput")
    wo = nc.dram_tensor("wo", [HD, E], BF16, kind="ExternalInput")
    bo = nc.dram_tensor("bo", [1, E], BF16, kind="ExternalInput")

    weiT = nc.dram_tensor("weiT", [H, S, SL], BF16, kind="ExternalOutput")
    out = nc.dram_tensor("out", [SL, E], F32, kind="ExternalOutput")

    Exp = mybir.ActivationFunctionType.Exp
    scale = 1.0 / np.sqrt(np.float32(D))

    with TileContext(nc) as tc:
        with (
            tc.tile_pool(name="persist", bufs=1) as persist,
            tc.tile_pool(name="work", bufs=2) as work,
            tc.tile_pool(name="psum", bufs=2, space="PSUM") as psum,
        ):
            # ---- constants ----
            ones_bf = persist.tile([1, 128], BF16, tag="ones_bf")
            nc.vector.memset(ones_bf[:, :], 1.0)
            ones_f32 = persist.tile([1, 128], F32, tag="ones_f32")
            nc.vector.memset(ones_f32[:, :], 1.0)
            bo_sb = persist.tile([1, E], BF16, tag="bo_sb")
            nc.sync.dma_start(bo_sb[:, :], bo[:, :])

            wo_sb = []
            for p in range(NP):
                t = persist.tile([128, E], BF16, tag=f"wo{p}", name=f"wo_sb{p}")
                nc.sync.dma_start(t[:, :], wo[128 * p : 128 * (p + 1), :])
                wo_sb.append(t)

            # persistent activations
            kt = [
                persist.tile([128, S], BF16, tag=f"kt{p}", name=f"kt{p}")
                for p in range(NP)
            ]
            qt = [
                persist.tile([128, SL], BF16, tag=f"qt{p}", name=f"qt{p}")
                for p in range(NP)
            ]
            v_sb = [
                persist.tile([128, H * 65], BF16, tag=f"v{tb}", name=f"v_sb{tb}")
                for tb in range(T128)
            ]
            catT = [
                persist.tile([128, SL], BF16, tag=f"cat{p}", name=f"catT{p}")
                for p in range(NP)
            ]

            # ---- phase 1a: V = x @ Wv for all heads, [t, hd] layout ----
            with tc.tile_pool(name="ph1", bufs=1) as ph1:
                xt = []
                for ec in range(EC):
                    t = ph1.tile([128, S], BF16, tag=f"xt{ec}", name=f"xt{ec}")
                    nc.sync.dma_start(t[:, :], xT[128 * ec : 128 * (ec + 1), :])
                    xt.append(t)
                xtq = []
                for ec in range(EC):
                    t = ph1.tile([128, SL], BF16, tag=f"xtq{ec}", name=f"xtq{ec}")
                    nc.sync.dma_start(t[:, :], xTq[128 * ec : 128 * (ec + 1), :])
                    xtq.append(t)

                with tc.tile_pool(name="wvp", bufs=1) as wvp:
                    wv_sb = []
                    for ec in range(EC):
                        t = wvp.tile([128, HD], BF16, tag=f"wv{ec}", name=f"wv_sb{ec}")
                        nc.sync.dma_start(
                            t[:, :], wv[128 * ec : 128 * (ec + 1), :]
                        )
                        wv_sb.append(t)

                    for tb in range(T128):
                        pv = psum.tile([128, HD], F32, tag="pv", bufs=1, name="pv")
                        for ec in range(EC):
                            for nb in range(NB):
                                nc.tensor.matmul(
                                    pv[:, VW * nb : VW * (nb + 1)],
                                    lhsT=xt[ec][:, 128 * tb : 128 * (tb + 1)],
                                    rhs=wv_sb[ec][:, VW * nb : VW * (nb + 1)],
                                    start=(ec == 0),
                                    stop=(ec == EC - 1),
                                )
                        vt = v_sb[tb]
                        vt3 = vt[:, :].rearrange("p (h c) -> p h c", c=65)
                        nc.vector.memset(vt3[:, :, 64:65], 1.0)
                        nc.vector.tensor_copy(
                            vt3[:, :, 0:64],
                            pv[:, :].rearrange("p (h c) -> p h c", c=64),
                        )

                # ---- phase 1b: K^T, Q^T per head pair ----
                with tc.tile_pool(name="wqkp", bufs=2) as wqkp:
                    for p in range(NP):
                        wkc, wqc = [], []
                        for ec in range(EC):
                            a = wqkp.tile([128, 128], BF16, tag=f"wkc{ec}",
                                          name=f"wkc{p}_{ec}")
                            nc.sync.dma_start(
                                a[:, :],
                                wk[128 * ec : 128 * (ec + 1),
                                   128 * p : 128 * (p + 1)],
                            )
                            wkc.append(a)
                            b = wqkp.tile([128, 128], BF16, tag=f"wqc{ec}",
                                          name=f"wqc{p}_{ec}")
                            nc.sync.dma_start(
                                b[:, :],
                                wq[128 * ec : 128 * (ec + 1),
                                   128 * p : 128 * (p + 1)],
                            )
                            wqc.append(b)

                        for tb in range(S // TW):
                            pk = psum.tile([128, TW], F32, tag="pkq", name="pk")
                            for ec in range(EC):
                                nc.tensor.matmul(
                                    pk[:, :],
                                    lhsT=wkc[ec][:, :],
                                    rhs=xt[ec][:, TW * tb : TW * (tb + 1)],
                                    start=(ec == 0),
                                    stop=(ec == EC - 1),
                                )
                            nc.vector.tensor_copy(
                                kt[p][:, TW * tb : TW * (tb + 1)], pk[:, :]
                            )
                        for sb in range(SL // SW):
                            pq = psum.tile([128, SW], F32, tag="pkq", name="pq")
                            for ec in range(EC):
                                nc.tensor.matmul(
                                    pq[:, :],
                                    lhsT=wqc[ec][:, :],
                                    rhs=xtq[ec][:, SW * sb : SW * (sb + 1)],
                                    start=(ec == 0),
                                    stop=(ec == EC - 1),
                                )
                            nc.vector.tensor_copy(
                                qt[p][:, SW * sb : SW * (sb + 1)], pq[:, :]
                            )

            # ---- phase 2: attention per head / s-block ----
            for p in range(NP):
                for h01 in range(2):
                    h = 2 * p + h01
                    r0, r1 = 64 * h01, 64 * h01 + 64
                    for sb in range(SL // SW):
                        av = psum.tile([128, SW], F32, tag="av", name="av")
                        pts = []
                        for tcb in range(T128):
                            ps = psum.tile([128, SW], F32, tag="ps", name="ps")
                            nc.tensor.matmul(
                                ps[:, :],
                                lhsT=kt[p][r0:r1, 128 * tcb : 128 * (tcb + 1)],
                                rhs=qt[p][r0:r1, SW * sb : SW * (sb + 1)],
                                start=True,
                                stop=True,
                                tile_position=(64 * h01, 0),
                            )
                            pt = work.tile([128, SW], BF16, tag=f"pt{tcb}",
                                           bufs=1, name=f"pt{tcb}")
                            nc.scalar.activation(pt[:, :], ps[:, :], Exp,
                                                 scale=float(scale))
                            nc.tensor.matmul(
                                av[0:65, :],
                                lhsT=v_sb[tcb][:, 65 * h : 65 * h + 65],
                                rhs=pt[:, :],
                                start=(tcb == 0),
                                stop=(tcb == T128 - 1),
                            )
                            pts.append(pt)

                        rden = work.tile([1, SW], F32, tag="rden", name="rden")
                        nc.vector.reciprocal(rden[:, :], av[64:65, :])
                        pd = psum.tile([128, SW], F32, tag="ps", name="pd")
                        nc.tensor.matmul(
                            pd[:, :],
                            lhsT=ones_f32[0:1, 0:128],
                            rhs=rden[:, :],
                            start=True,
                            stop=True,
                        )
                        rbc = work.tile([128, SW], BF16, tag="rbc", name="rbc")
                        nc.vector.tensor_copy(rbc[:, :], pd[:, :])

                        for tcb in range(T128):
                            nc.vector.tensor_mul(
                                pts[tcb][:, :], pts[tcb][:, :], rbc[:, :]
                            )
                            nc.sync.dma_start(
                                weiT[h, 128 * tcb : 128 * (tcb + 1),
                                     SW * sb : SW * (sb + 1)],
                                pts[tcb][:, :],
                            )
                        nc.vector.tensor_mul(
                            catT[p][r0:r1, SW * sb : SW * (sb + 1)],
                            av[0:64, :],
                            rbc[0:64, :],
                        )

            # ---- phase 3: out = cat @ Wo + bo ----
            EB = max(1, E // 512)
            EW = min(512, E)
            for sb2 in range(SL // 128):
                for eb in range(EB):
                    po = psum.tile([128, EW], F32, tag="pkq", name="po")
                    for p in range(NP):
                        nc.tensor.matmul(
                            po[:, :],
                            lhsT=catT[p][:, 128 * sb2 : 128 * (sb2 + 1)],
                            rhs=wo_sb[p][:, EW * eb : EW * (eb + 1)],
                            start=(p == 0),
                            stop=False,
                        )
                    nc.tensor.matmul(
                        po[:, :],
                        lhsT=ones_bf[0:1, 0:128],
                        rhs=bo_sb[0:1, EW * eb : EW * (eb + 1)],
                        start=False,
                        stop=True,
                    )
                    osb = work.tile([128, EW], F32, tag="osb", name="osb")
                    nc.vector.tensor_copy(osb[:, :], po[:, :])
                    nc.sync.dma_start(
                        out[128 * sb2 : 128 * (sb2 + 1),
                            EW * eb : EW * (eb + 1)],
                        osb[:, :],
                    )

    if split_waits:
        _split_multi_waits(nc)
    return nc


# ----------------------------------------------------------------------------
# Host-side entry point: full inputs in, full outputs out.
# ----------------------------------------------------------------------------

_NC_CACHE = {}


def _get_program(E, H, S, SL):
    key = (E, H, S, SL)
    if key not in _NC_CACHE:
        _NC_CACHE[key] = build_core_program(E=E, H=H, S=S, SL=SL)
    return _NC_CACHE[key]


def _bf16(a):
    return np.asarray(a).astype(ml_dtypes.bfloat16)


def kernel(x, Wq, Wk, Wv, Wo, bo, _collect_results=None):
    B, S, E = x.shape
    H = Wq.shape[0]
    D = Wq.shape[2]
    assert D == 64
    n_cores = 8
    splits_per_batch = n_cores // B
    SL = S // splits_per_batch

    nc = _get_program(E, H, S, SL)

    # host-side shared weight prep
    wq_r = _bf16(np.transpose(np.asarray(Wq), (1, 0, 2)).reshape(E, H * D))
    wk_r = _bf16(np.transpose(np.asarray(Wk), (1, 0, 2)).reshape(E, H * D))
    wv_r = _bf16(np.transpose(np.asarray(Wv), (1, 0, 2)).reshape(E, H * D))
    wo_r = _bf16(np.asarray(Wo))
    bo_r = _bf16(np.asarray(bo).reshape(1, E))

    in_maps = []
    for c in range(n_cores):
        b = c // splits_per_batch
        s0 = (c % splits_per_batch) * SL
        xTb = np.ascontiguousarray(_bf16(np.asarray(x[b])).T)  # [E, S]
        in_maps.append(
            {
                "xT": xTb,
                "xTq": np.ascontiguousarray(xTb[:, s0 : s0 + SL]),
                "wq": wq_r,
                "wk": wk_r,
                "wv": wv_r,
                "wo": wo_r,
                "bo": bo_r,
            }
        )

    res = bass_utils.run_bass_kernel_spmd(nc, in_maps, core_ids=list(range(n_cores)))
    if _collect_results is not None:
        _collect_results.append(res)

    out = np.empty((B, S, E), dtype=np.float32)
    wei = np.empty((H, B, S, S), dtype=np.float32)
    for c in range(n_cores):
        b = c // splits_per_batch
        s0 = (c % splits_per_batch) * SL
        r = res.results[c]
        out[b, s0 : s0 + SL, :] = r["out"]
        # weiT is [H, S(t), SL(s)] bf16 -> wei[h, b, s, t]
        wei[:, b, s0 : s0 + SL, :] = np.transpose(
            r["weiT"].astype(np.float32), (0, 2, 1)
        )
    return out, wei


# revision 9
# speedup vs baseline: 1.3233x; 1.3233x over previous
"""Multi-head attention (16 heads, B=4, S=2048, E=1024) on 8 Trainium2 cores.

Sharding: core c handles batch b = c//2 and the s-half (c%2) of the query
rows — 1024 query rows per core, all 16 heads.  K/V are computed for the
full sequence of the core's batch on-chip (small recompute, no collectives).

Orientation: everything is kept transposed ([t, s] / [d, s] / [hd, s]) so
that every matmul contraction lands on the partition axis:
  K^T[d,t], Q^T[d,s]  -> scores^T[t,s]: the two heads of a pair run as
    row-tiled concurrent matmuls into one 2-bank psum tile [t, (h01 s)]
  one exp per pair/t-block on ACT (scale=1/sqrt(D) folded in) -> P^T bf16
  AV: lhsT = V|ones [t, 65] -> psum[0:64]=head_out^T raw, psum[64]=denom
  denom -> bf16 reciprocal (DVE) -> broadcast over partitions via a
    ones-matmul on the PE -> rbc
  wei^T = P^T * rbc (DVE) -> DMA out in contiguous tile-major layout
    (bf16); the host untiles/transposes/casts during unsharding
  out = cat^T.T @ Wo + ones.T @ bo (PE accumulation, bias folded in)
"""

import numpy as np
import ml_dtypes

import concourse.bass as bass
import concourse.mybir as mybir
from concourse.tile import TileContext
from concourse import bass_utils

BF16 = mybir.dt.bfloat16
F32 = mybir.dt.float32

# ----------------------------------------------------------------------------
# Workarounds for this container's pinned toolchain.
# ----------------------------------------------------------------------------


def _patch_tile_drain():
    """walrus here rejects >1 sync-wait on the Tile tail drain (CTRL encoding
    limit).  Split the waits across several drain instructions on the sync
    engine — they serialize, so semantics are unchanged."""
    import concourse.tile as ctile
    from concourse.vector_clock import ScopedClock

    if getattr(ctile.TileContext, "_drain_split_patched", False):
        return

    def _drain_and_barrier_split(self, tick_clock, wait_clock):
        drain_inst = self.nc.sync.drain()
        wait_clock.add_sem_waits(
            drain_inst.ins, ScopedClock({None: tick_clock.global_clock})
        )
        waits = drain_inst.ins.sync_info.on_wait
        if len(waits) > 1:
            extra = list(waits[1:])
            del waits[1:]
            for sw in extra:
                nxt = self.nc.sync.drain()
                nxt.ins.sync_info = mybir.SyncInfo(on_wait=[sw], on_update=[])

        self.nc.all_engine_barrier()
        assert self.sems is not None
        popped = self.nc._tile_sem_poison_stack.pop()
        assert popped is self._sem_poison
        self.nc.clear_and_free_semaphores(list(self.sems.allocated().values()))
        self.nc.all_engine_barrier()

    ctile.TileContext._drain_and_barrier = _drain_and_barrier_split
    ctile.TileContext._drain_split_patched = True


_patch_tile_drain()


def _split_multi_waits(nc):
    """This walrus build accepts at most one sync-wait per instruction.
    Hoist extra waits onto single-wait carrier instructions inserted
    immediately before the instruction on the same engine — identical
    blocking semantics, engine order preserved.  DVE/Pool use ENGINE_NOP
    (159), ACT uses NOP (164); PE/SP use drains (rare there)."""
    k = 0
    for f in nc.m.functions:
        for bb in f.blocks:
            new_list = []
            for inst in bb.instructions:
                si = getattr(inst, "sync_info", None)
                if si is not None and si.on_wait and len(si.on_wait) > 1:
                    if inst.engine in (mybir.EngineType.DVE, mybir.EngineType.Pool):
                        opv = 159
                    elif inst.engine == mybir.EngineType.Activation:
                        opv = 164
                    else:
                        opv = None
                    waits = list(si.on_wait)
                    for sw in waits[:-1]:
                        k += 1
                        if opv is not None:
                            carrier = mybir.InstISA(
                                name=f"wsplit-{k}",
                                isa_opcode=opv,
                                op_name="ENGINE_NOP" if opv == 159 else "NOP",
                                instr=[opv, 16] + [0] * 62,
                                ant_dict={
                                    "header": {"opcode": opv, "inst_word_len": 16}
                                },
                                ant_isa_is_sequencer_only=False,
                                engine=inst.engine,
                                ins=[],
                                outs=[],
                                sync_info=mybir.SyncInfo(
                                    on_wait=[sw], on_update=[]
                                ),
                            )
                        else:
                            carrier = mybir.InstDrain(
                                name=f"wsplit-{k}",
                                engine=inst.engine,
                                ins=[],
                                outs=[],
                                sync_info=mybir.SyncInfo(
                                    on_wait=[sw], on_update=[]
                                ),
                            )
                        new_list.append(carrier)
                    inst.sync_info = mybir.SyncInfo(
                        on_wait=[waits[-1]],
                        on_update=list(si.on_update or []),
                    )
                new_list.append(inst)
            bb.instructions = new_list
    return k


# ----------------------------------------------------------------------------
# Per-core Bass program
# ----------------------------------------------------------------------------


def build_core_program(E=1024, H=16, S=2048, SL=1024, split_waits=True):
    """Emit the per-core MHA program.  All cores run this same program on
    different input data (SPMD).  D is fixed at 64.

    Per-core inputs (bf16):
      xT   [E, S]   - x[b].T, full sequence of this core's batch
      xTq  [E, SL]  - local query columns of xT
      wq/wk/wv [E, H*64] - per-head projection weights, hd = h*64+d
      wo   [H*64, E]
      bo   [1, E]
    Outputs:
      weiT [H, SB, TCG, 128, TGRP*SW] bf16 - attention map, tile-major:
           value [h, sb, tcg, p, j*SW+sc] = wei[h, s=sb*SW+sc,
                                                t=tcg*TGRP*128 + j*128 + p]
      out  [SL, E] f32
    """
    D = 64
    HD = H * D
    EC = E // 128          # e-chunks (contraction blocks for projections)
    NP = H // 2            # head pairs
    T128 = S // 128        # key/t blocks of 128
    TGRP = 4               # t-blocks per wei DMA group
    TCG = T128 // TGRP
    TW = min(512, S)       # t width for K^T production
    SW = min(512, SL)      # s width for scores / attention blocks
    SB = SL // SW
    VW = min(512, HD)
    NB = HD // VW

    nc = bass.Bass(trn_type="TRN2")

    xT = nc.dram_tensor("xT", [E, S], BF16, kind="ExternalInput")
    xTq = nc.dram_tensor("xTq", [E, SL], BF16, kind="ExternalInput")
    wq = nc.dram_tensor("wq", [E, HD], BF16, kind="ExternalInput")
    wk = nc.dram_tensor("wk", [E, HD], BF16, kind="ExternalInput")
    wv = nc.dram_tensor("wv", [E, HD], BF16, kind="ExternalInput")
    wo = nc.dram_tensor("wo", [HD, E], BF16, kind="ExternalInput")
    bo = nc.dram_tensor("bo", [1, E], BF16, kind="ExternalInput")

    weiT = nc.dram_tensor(
        "weiT", [H, SB, TCG, 128, TGRP * SW], BF16, kind="ExternalOutput"
    )
    out = nc.dram_tensor("out", [SL, E], F32, kind="ExternalOutput")

    Exp = mybir.ActivationFunctionType.Exp
    scale = 1.0 / np.sqrt(np.float32(D))

    with TileContext(nc) as tc:
        with (
            tc.tile_pool(name="persist", bufs=1) as persist,
            tc.tile_pool(name="work", bufs=2) as work,
            tc.tile_pool(name="psum", bufs=2, space="PSUM") as psum,
        ):
            # ---- constants ----
            ones_bf = persist.tile([1, 128], BF16, tag="ones_bf")
            nc.vector.memset(ones_bf[:, :], 1.0)
            bo_sb = persist.tile([1, E], BF16, tag="bo_sb")
            nc.sync.dma_start(bo_sb[:, :], bo[:, :])

            # persistent activations
            kt = [
                persist.tile([128, S], BF16, tag=f"kt{p}", name=f"kt{p}")
                for p in range(NP)
            ]
            qt = [
                persist.tile([128, SL], BF16, tag=f"qt{p}", name=f"qt{p}")
                for p in range(NP)
            ]
            v_sb = [
                persist.tile([128, H * 65], BF16, tag=f"v{tb}", name=f"v_sb{tb}")
                for tb in range(T128)
            ]
            catT = [
                persist.tile([128, SL], BF16, tag=f"cat{p}", name=f"catT{p}")
                for p in range(NP)
            ]

            with tc.tile_pool(name="ph1", bufs=1) as ph1:
                xt = []
                for ec in range(EC):
                    t = ph1.tile([128, S], BF16, tag=f"xt{ec}", name=f"xt{ec}")
                    nc.sync.dma_start(t[:, :], xT[128 * ec : 128 * (ec + 1), :])
                    xt.append(t)
                xtq = []
                for ec in range(EC):
                    t = ph1.tile([128, SL], BF16, tag=f"xtq{ec}", name=f"xtq{ec}")
                    nc.sync.dma_start(t[:, :], xTq[128 * ec : 128 * (ec + 1), :])
                    xtq.append(t)

                # ---- phase 1a: V = x @ Wv for all heads, [t, hd] layout ----
                with tc.tile_pool(name="wvp", bufs=1) as wvp:
                    wv_sb = []
                    for ec in range(EC):
                        t = wvp.tile([128, HD], BF16, tag=f"wv{ec}", name=f"wv_sb{ec}")
                        nc.sync.dma_start(t[:, :], wv[128 * ec : 128 * (ec + 1), :])
                        wv_sb.append(t)

                    for tb in range(T128):
                        pv = psum.tile([128, HD], F32, tag="pv", bufs=1, name="pv")
                        for ec in range(EC):
                            for nb in range(NB):
                                nc.tensor.matmul(
                                    pv[:, VW * nb : VW * (nb + 1)],
                                    lhsT=xt[ec][:, 128 * tb : 128 * (tb + 1)],
                                    rhs=wv_sb[ec][:, VW * nb : VW * (nb + 1)],
                                    start=(ec == 0),
                                    stop=(ec == EC - 1),
                                )
                        vt = v_sb[tb]
                        vt3 = vt[:, :].rearrange("p (h c) -> p h c", c=65)
                        nc.vector.memset(vt3[:, :, 64:65], 1.0)
                        nc.vector.tensor_copy(
                            vt3[:, :, 0:64],
                            pv[:, :].rearrange("p (h c) -> p h c", c=64),
                        )

                # ---- interleaved: per pair, K^T/Q^T production + attention ----
                with tc.tile_pool(name="wqkp", bufs=2) as wqkp:
                    for p in range(NP):
                        wkc, wqc = [], []
                        for ec in range(EC):
                            a = wqkp.tile([128, 128], BF16, tag=f"wkc{ec}",
                                          name=f"wkc{p}_{ec}")
                            nc.sync.dma_start(
                                a[:, :],
                                wk[128 * ec : 128 * (ec + 1),
                                   128 * p : 128 * (p + 1)],
                            )
                            wkc.append(a)
                            b = wqkp.tile([128, 128], BF16, tag=f"wqc{ec}",
                                          name=f"wqc{p}_{ec}")
                            nc.sync.dma_start(
                                b[:, :],
                                wq[128 * ec : 128 * (ec + 1),
                                   128 * p : 128 * (p + 1)],
                            )
                            wqc.append(b)

                        for tb in range(S // TW):
                            pk = psum.tile([128, TW], F32, tag="av", name="pk")
                            for ec in range(EC):
                                nc.tensor.matmul(
                                    pk[:, :],
                                    lhsT=wkc[ec][:, :],
                                    rhs=xt[ec][:, TW * tb : TW * (tb + 1)],
                                    start=(ec == 0),
                                    stop=(ec == EC - 1),
                                )
                            nc.vector.tensor_copy(
                                kt[p][:, TW * tb : TW * (tb + 1)], pk[:, :]
                            )
                        for sb in range(SL // SW):
                            pq = psum.tile([128, SW], F32, tag="av", name="pq")
                            for ec in range(EC):
                                nc.tensor.matmul(
                                    pq[:, :],
                                    lhsT=wqc[ec][:, :],
                                    rhs=xtq[ec][:, SW * sb : SW * (sb + 1)],
                                    start=(ec == 0),
                                    stop=(ec == EC - 1),
                                )
                            nc.vector.tensor_copy(
                                qt[p][:, SW * sb : SW * (sb + 1)], pq[:, :]
                            )

                        # ---- attention for this pair ----
                        for sb in range(SB):
                            av01 = [
                                psum.tile([128, SW], F32, tag="av",
                                          name=f"av{h01}")
                                for h01 in range(2)
                            ]
                            ptgs = [
                                work.tile([128, 2 * TGRP * SW], BF16,
                                          tag=f"ptg{g}", bufs=1, name=f"ptg{g}")
                                for g in range(TCG)
                            ]
                            for tcb in range(T128):
                                g, j = tcb // TGRP, tcb % TGRP
                                ps = psum.tile([128, 2 * SW], F32, tag="ps",
                                               name="ps")
                                for h01 in range(2):
                                    nc.tensor.matmul(
                                        ps[:, SW * h01 : SW * (h01 + 1)],
                                        lhsT=kt[p][64 * h01 : 64 * h01 + 64,
                                                   128 * tcb : 128 * (tcb + 1)],
                                        rhs=qt[p][64 * h01 : 64 * h01 + 64,
                                                  SW * sb : SW * (sb + 1)],
                                        start=True,
                                        stop=True,
                                        tile_position=(64 * h01, 0),
                                    )
                                ptg = ptgs[g]
                                po4 = ptg[:, :].rearrange(
                                    "p (h j s) -> p h j s", h=2, j=TGRP
                                )
                                nc.scalar.activation(
                                    po4[:, :, j, :],
                                    ps[:, :].rearrange("p (h s) -> p h s", h=2),
                                    Exp,
                                    scale=float(scale),
                                )
                                for h01 in range(2):
                                    nc.tensor.matmul(
                                        av01[h01][0:65, :],
                                        lhsT=v_sb[tcb][:, 65 * (2 * p + h01):
                                                       65 * (2 * p + h01) + 65],
                                        rhs=ptg[:, (h01 * TGRP + j) * SW:
                                                (h01 * TGRP + j + 1) * SW],
                                        start=(tcb == 0),
                                        stop=(tcb == T128 - 1),
                                    )

                            for h01 in range(2):
                                h = 2 * p + h01
                                av = av01[h01]
                                rdenb = work.tile([1, SW], BF16, tag="rdenb",
                                                  name="rdenb")
                                with nc.allow_low_precision("softmax recip bf16"):
                                    nc.vector.reciprocal(
                                        rdenb[:, :], av[64:65, :]
                                    )
                                pd = psum.tile([128, SW], F32, tag="ps",
                                               name="pd")
                                nc.tensor.matmul(
                                    pd[:, :],
                                    lhsT=ones_bf[0:1, 0:128],
                                    rhs=rdenb[:, :],
                                    start=True,
                                    stop=True,
                                )
                                rbc = work.tile([128, SW], BF16, tag="rbc",
                                                name="rbc")
                                nc.vector.tensor_copy(rbc[:, :], pd[:, :])

                                for g in range(TCG):
                                    ptg = ptgs[g]
                                    for j in range(TGRP):
                                        sl = ptg[:, (h01 * TGRP + j) * SW:
                                                 (h01 * TGRP + j + 1) * SW]
                                        nc.vector.tensor_mul(sl, sl, rbc[:, :])
                                    nc.sync.dma_start(
                                        weiT[h, sb, g, :, :],
                                        ptg[:, h01 * TGRP * SW:
                                            (h01 + 1) * TGRP * SW],
                                    )
                                nc.vector.tensor_mul(
                                    catT[p][64 * h01 : 64 * h01 + 64,
                                            SW * sb : SW * (sb + 1)],
                                    av[0:64, :],
                                    rbc[0:64, :],
                                )

            # ---- phase 3: out = cat @ Wo + bo ----
            with tc.tile_pool(name="ph3", bufs=1) as ph3:
                wo_sb = []
                for p in range(NP):
                    t = ph3.tile([128, E], BF16, tag=f"wo{p}", name=f"wo_sb{p}")
                    nc.sync.dma_start(t[:, :], wo[128 * p : 128 * (p + 1), :])
                    wo_sb.append(t)

                EW = min(512, E)
                EB = E // EW
                for sb2 in range(SL // 128):
                    for eb in range(EB):
                        po = psum.tile([128, EW], F32, tag="av", name="po")
                        for p in range(NP):
                            nc.tensor.matmul(
                                po[:, :],
                                lhsT=catT[p][:, 128 * sb2 : 128 * (sb2 + 1)],
                                rhs=wo_sb[p][:, EW * eb : EW * (eb + 1)],
                                start=(p == 0),
                                stop=False,
                            )
                        nc.tensor.matmul(
                            po[:, :],
                            lhsT=ones_bf[0:1, 0:128],
                            rhs=bo_sb[0:1, EW * eb : EW * (eb + 1)],
                            start=False,
                            stop=True,
                        )
                        osb = work.tile([128, EW], F32, tag="osb", name="osb")
                        nc.vector.tensor_copy(osb[:, :], po[:, :])
                        nc.sync.dma_start(
                            out[128 * sb2 : 128 * (sb2 + 1),
                                EW * eb : EW * (eb + 1)],
                            osb[:, :],
                        )

    if split_waits:
        _split_multi_waits(nc)
    return nc


# ----------------------------------------------------------------------------
# Host-side entry point: full inputs in, full outputs out.
# ----------------------------------------------------------------------------

_NC_CACHE = {}


def _get_program(E, H, S, SL):
    key = (E, H, S, SL)
    if key not in _NC_CACHE:
        _NC_CACHE[key] = build_core_program(E=E, H=H, S=S, SL=SL)
    return _NC_CACHE[key]


def _bf16(a):
    return np.asarray(a).astype(ml_dtypes.bfloat16)


def kernel(x, Wq, Wk, Wv, Wo, bo, _collect_results=None):
    B, S, E = x.shape
    H = Wq.shape[0]
    D = Wq.shape[2]
    assert D == 64
    n_cores = 8
    splits_per_batch = n_cores // B
    SL = S // splits_per_batch
    TGRP = 4
    SW = min(512, SL)
    SB = SL // SW
    TCG = (S // 128) // TGRP

    nc = _get_program(E, H, S, SL)

    # host-side shared weight prep
    wq_r = _bf16(np.transpose(np.asarray(Wq), (1, 0, 2)).reshape(E, H * D))
    wk_r = _bf16(np.transpose(np.asarray(Wk), (1, 0, 2)).reshape(E, H * D))
    wv_r = _bf16(np.transpose(np.asarray(Wv), (1, 0, 2)).reshape(E, H * D))
    wo_r = _bf16(np.asarray(Wo))
    bo_r = _bf16(np.asarray(bo).reshape(1, E))

    in_maps = []
    for c in range(n_cores):
        b = c // splits_per_batch
        s0 = (c % splits_per_batch) * SL
        xTb = np.ascontiguousarray(_bf16(np.asarray(x[b])).T)  # [E, S]
        in_maps.append(
            {
                "xT": xTb,
                "xTq": np.ascontiguousarray(xTb[:, s0 : s0 + SL]),
                "wq": wq_r,
                "wk": wk_r,
                "wv": wv_r,
                "wo": wo_r,
                "bo": bo_r,
            }
        )

    res = bass_utils.run_bass_kernel_spmd(nc, in_maps, core_ids=list(range(n_cores)))
    if _collect_results is not None:
        _collect_results.append(res)

    out = np.empty((B, S, E), dtype=np.float32)
    wei = np.empty((H, B, S, S), dtype=np.float32)
    for c in range(n_cores):
        b = c // splits_per_batch
        s0 = (c % splits_per_batch) * SL
        r = res.results[c]
        out[b, s0 : s0 + SL, :] = r["out"]
        # weiT tile-major [h, sb, tcg, p, (j sc)] -> wei[h, b, s, t]
        w = r["weiT"].reshape(H, SB, TCG, 128, TGRP, SW)
        wei[:, b, s0 : s0 + SL, :] = (
            w.transpose(0, 1, 5, 2, 4, 3)
            .reshape(H, SL, S)
            .astype(np.float32)
        )
    return out, wei
